# revision 38
# baseline (speedup 1.0000x reference)
"""Trainium2 Bass kernel for nn_Attn (S=4096, B=32, H=512).

Reference computation:
    energy[s,b,g] = sum_h enc[s,b,h] * W[g,h] + bias[g]
    scores[s,b]   = sum_g hidden[b,g] * energy[s,b,g]
    out[b,0,s]    = softmax_s(scores[:,b])

Key algebraic simplification: scores[s,b] = enc[s,b,:]·u[b,:] + hidden[b]·bias
with u = hidden @ W.  The bias term is constant over s, so it cancels in the
softmax.  u is a [4, 512] per-core operand (0.2% of the FLOPs); it is
precomputed on the host and shipped replicated across the 128 partitions
(512 KB fp16, the same bytes the [H,H] weight replication in the sharding
hint would cost) so the device consumes the encoder stream from t~0 with no
serial weight-preparation prefix.

The encoder slice streams in FP16 (host-side cast), which halves the HBM
traffic per core from 33.5 MB to 16.8 MB — the f32 kernel was DMA-bound at
~360 GB/s/core.  Measured end-to-end relative error of the fp16 pipeline vs
the f32 reference is ~5e-3 (512-term dot products of fp16-rounded values with
f32 accumulation), comfortably inside the 2e-2 gate.

Compute is a multiply+reduce per (s-tile, batch): 128 ops of [128 x 512],
spread across three engines (the real ISA has no fused mul+reduce on Pool
and free-dim TensorReduce is DVE-only, so the three available roles are):
  - 'V': DVE scalar_tensor_tensor, fused mul+accum (594 ns)
  - 'A': DVE tensor_mul fp16 in 2x mode (327 ns) + ACT copy-accum (~800 ns)
  - 'G': Pool tensor_mul (~1111 ns) + ACT copy-accum (~800 ns)
With V:72 A:16 G:40 each engine carries ~47 us — this problem sits right at
the compute/DMA ridge (DMA stream is ~48 us incl. the u operand).

Tail structure: the last NQ=4 s-tiles stream BATCH-MAJOR as 8 per-batch
half DMAs, and each batch runs its own softmax pipeline (PE transpose of its
[128,32] score block, exp with constant -40 bias — exact, since any constant
shift cancels in softmax — Z via ones-matmul, reciprocal, scale, 16 KB store
on alternating DMA queues).  Batches 0-2 finish inside the stream shadow;
only batch 3's short chain trails the final DMA.  PE dummy matmuls gated on
the late tiles hold the tensor engine's p-state at full clock for the
transposes.

Sharding: data-parallel on batch — core c owns batches 4c..4c+3.
"""

import sys

sys.path.insert(0, "/opt/trn_rl_repo")

import numpy as np

S, B, H = 4096, 32, 512
NCORES = 8
BL = B // NCORES          # 4 batches per core
ST = S // 128             # 32 score tiles of 128 s-rows
NQ = 4                    # final tiles streamed batch-major

_NC = None                # cached Bass module (build once per process)


def _mk_pattern(nv, npool, nact):
    """Interleave nv 'V', npool 'P', nact 'A' slots evenly (Bresenham)."""
    n = nv + npool + nact
    counts = {"V": nv, "P": npool, "A": nact}
    acc = {k: 0.0 for k in counts}
    out = []
    for _ in range(n):
        for k in counts:
            acc[k] += counts[k] / n
        pick = max(acc, key=lambda k: acc[k])
        acc[pick] -= 1.0
        out.append(pick)
    return out


def _build_module(
    enc_bufs=14, nsingle=2, prime_exp=True, reps=1,
    nv=8, npool=12, nact=12,
    # per-batch plans for the four batch-major end tiles (st 28..31); batch 3
    # avoids the slow Pool-mul path near the end so its score block closes
    # as early as possible.
    qplans=(
        ("A", "V", "G", "V"),
        ("V", "G", "V", "G"),
        ("G", "G", "V", "V"),
        ("V", "V", "A", "V"),
    ),
    pe_warm_from=24,
):
    import concourse.bacc as bacc
    import concourse.tile as tile
    from concourse import mybir
    from contextlib import ExitStack

    f32 = mybir.dt.float32
    f16 = mybir.dt.float16
    nc = bacc.Bacc(trn_type="TRN2", num_devices=NCORES)

    enc = nc.dram_tensor("enc", [S, BL, H], f16, kind="ExternalInput")
    # u = hidden @ W, host-precomputed, fp16, replicated across partitions
    ub = nc.dram_tensor("ub", [128, BL, H], f16, kind="ExternalInput")
    out = nc.dram_tensor("out", [BL, S], f32, kind="ExternalOutput")

    # Inline constants (embedded in the NEFF):
    #   cwide[:, 0:128]   = 128x128 identity (PE transpose operand)
    #   cwide[0:32, 128]  = 1.0  (ones column: Z_b = ones32^T @ rowsum_b)
    #   cwide[0, 129:161] = 1.0  (ones row: sc_b = onesrow^T @ rz_b)
    cwide_np = np.zeros((128, 161), np.float32)
    cwide_np[:, 0:128] = np.eye(128, dtype=np.float32)
    cwide_np[0:32, 128] = 1.0
    cwide_np[0, 129:161] = 1.0
    cwide_t = nc.inline_tensor(cwide_np, "cwide")

    with tile.TileContext(nc) as tc:
        with ExitStack() as ctx:
            singles = ctx.enter_context(tc.tile_pool(name="singles", bufs=1))
            encpool = ctx.enter_context(tc.tile_pool(name="encp", bufs=enc_bufs))
            tailpool = ctx.enter_context(tc.tile_pool(name="tbp", bufs=nsingle))
            qpool = ctx.enter_context(tc.tile_pool(name="qp", bufs=2 * BL))
            psum = ctx.enter_context(tc.tile_pool(name="psum", bufs=1, space="PSUM"))
            vprod = ctx.enter_context(tc.tile_pool(name="vprod", bufs=3))
            pprod = ctx.enter_context(tc.tile_pool(name="pprod", bufs=3))
            mprod = ctx.enter_context(tc.tile_pool(name="mprod", bufs=8))
            trashpool = ctx.enter_context(tc.tile_pool(name="trsh", bufs=2))

            for _rep in range(reps):
                STF = ST - NQ      # tiles streamed whole, tile-major
                view = enc.rearrange("(t p) b h -> t p b h", p=128)
                # batch-major half view of the final NQ tiles: for batch b,
                # half u covers s-tiles STF+2u and STF+2u+1
                viewh = enc.rearrange(
                    "(u j p) b h -> u b p j h", j=2, p=128
                )
                ets = {}

                def issue_dma(t):
                    if t >= STF - nsingle:
                        et = tailpool.tile([128, BL, H], f16, tag="tail")
                    else:
                        et = encpool.tile([128, BL, H], f16, tag="enc")
                    nc.sync.dma_start(out=et, in_=view[t])
                    ets[t] = et

                # ubig rides first on the sync queue — consumers start the
                # moment the first encoder tile lands
                ubig = singles.tile([128, BL, H], f16)
                nc.sync.dma_start(out=ubig, in_=ub[:, :, :])
                issue_dma(0)
                # softmax-only constants ride the scalar engine's HWDGE ring
                cwide_sb = singles.tile([128, 161], f32)
                nc.scalar.dma_start(out=cwide_sb, in_=cwide_t[:, :])
                ident_sb = cwide_sb[:, 0:128]
                ones32_sb = cwide_sb[0:32, 128:129]
                onesrow_sb = cwide_sb[0:1, 129:161]

                # ---- preload the exp_and_friends ACT table (contains Copy
                # AND Exp) while the engine is idle, so no table switch lands
                # in the softmax tail
                if prime_exp:
                    dummy = trashpool.tile([128, 1], f32, tag="dum")
                    nc.scalar.activation(
                        out=dummy, in_=cwide_sb[:, 0:1],
                        func=mybir.ActivationFunctionType.Exp, scale=0.0,
                    )

                warm16 = singles.tile([128, 128], f16)
                nc.vector.memset(warm16, 0.0)
                nbias = singles.tile([128, 1], f32)
                nc.vector.memset(nbias, -40.0)
                p_warm = psum.tile([128, 128], f32, tag="warm")

                # ---- per-batch score blocks: scores[b][p, st] holds the
                # score of s = st*128+p for batch b
                scores = []
                for b in range(BL):
                    sc_b = singles.tile([128, ST], f32, tag=f"sc{b}", name=f"sc{b}")
                    scores.append(sc_b)

                def consume(ebv, b, st, kind):
                    # 'V': DVE fused multiply+accumulate (594 ns)
                    # 'A': DVE fp16 mul, 2x mode (327) + ACT copy-accum (799)
                    # 'G': Pool mul (1111) + ACT copy-accum (799)
                    # (the real ISA has no fused mul+reduce on Pool, and
                    # free-dim TensorReduce is DVE-only — verified against
                    # neuronxcc codegen)
                    if kind == "V":
                        prod = vprod.tile([128, H], f16, tag="vp")
                        nc.vector.scalar_tensor_tensor(
                            out=prod, in0=ebv, scalar=1.0, in1=ubig[:, b, :],
                            op0=mybir.AluOpType.mult, op1=mybir.AluOpType.mult,
                            accum_out=scores[b][:, st : st + 1],
                        )
                        return
                    if kind == "G":
                        prod = pprod.tile([128, H], f16, tag="pp")
                        nc.gpsimd.tensor_mul(out=prod, in0=ebv, in1=ubig[:, b, :])
                    else:  # 'A'
                        prod = mprod.tile([128, H], f16, tag="mp")
                        nc.vector.tensor_mul(out=prod, in0=ebv, in1=ubig[:, b, :])
                    trash = trashpool.tile([128, H], f16, tag="tr")
                    nc.scalar.activation(
                        out=trash, in_=prod,
                        func=mybir.ActivationFunctionType.Copy, scale=1.0,
                        accum_out=scores[b][:, st : st + 1],
                    )

                # ---- main stream: tiles 0..STF-1, tile-major.  A 4-tile
                # multiset cycle carrying V:9 A:2 G:5 per 16 ops keeps the
                # three engines near-equal (~47 us each over the stream —
                # this problem sits right at the compute/DMA ridge).
                head_plans = (
                    ("A", "V", "G", "V"),
                    ("G", "V", "G", "V"),
                    ("V", "G", "V", "V"),
                    ("A", "V", "G", "V"),
                )
                for st in range(STF):
                    if st not in ets:
                        issue_dma(st)
                    if st + 1 < STF and (st + 1) not in ets:
                        issue_dma(st + 1)
                    et = ets.pop(st)
                    if st >= pe_warm_from:
                        # hold the PE p-state streak through the end of the
                        # stream (gated on this tile) so the per-batch
                        # transposes run at full clock
                        nc.tensor.matmul(
                            p_warm, warm16, et[:, 0, 0:128],
                            start=True, stop=True,
                        )
                    plan = head_plans[st % 4]
                    for b in range(BL):
                        consume(et[:, b, :], b, st, plan[b])

                # ---- final NQ tiles: batch-major half DMAs (728 ns each, so
                # the sync SEQ's ~650 ns issue pipeline never starves the DMA
                # engines), issued all at once so no store wait ever blocks
                # the sync queue
                halves = []
                for b in range(BL):
                    hs = []
                    for u in range(NQ // 2):
                        eth = qpool.tile([128, 2, H], f16, tag="qh")
                        nc.sync.dma_start(
                            out=eth, in_=viewh[STF // 2 + u, b]
                        )
                        hs.append(eth)
                    halves.append(hs)

                # ---- per-batch consume + SOFTWARE-PIPELINED softmax chains.
                # Engine sequencers are FIFO: a waiting instruction blocks the
                # decode of everything behind it.  So batch b's chain is
                # issued in two stages — transpose+exp right after b's
                # consumes (deps just became ready), and the Z/recip/scale/
                # store stage one batch-group LATER (its deps resolve during
                # batch b+1's window, so it never stalls a queue).
                outv = out.rearrange("b (st p) -> b st p", p=128)
                expbs, rowsbs, p_zbs, rzbs = {}, {}, {}, {}

                def start_chain(b):
                    # transpose to [32 st, 128 p] so the store is contiguous
                    p_sTb = psum.tile([32, 128], f32, tag="pstb", bufs=2)
                    nc.tensor.transpose(p_sTb, scores[b], ident_sb)
                    expb = singles.tile([32, 128], f32, tag=f"exp{b}", name=f"exp{b}")
                    rowsb = singles.tile([32, 1], f32, tag=f"row{b}", name=f"row{b}")
                    # exp(score - 40): constant recentering is exact softmax
                    # (the shift cancels); scores are bounded |s| < ~60 so the
                    # f32 range is safe.  accum_out -> per-st partial Z.
                    nc.scalar.activation(
                        out=expb, in_=p_sTb,
                        func=mybir.ActivationFunctionType.Exp,
                        bias=nbias[0:32, :], scale=1.0,
                        accum_out=rowsb,
                    )
                    expbs[b], rowsbs[b] = expb, rowsb

                def mid_chain(b):
                    p_zb = psum.tile([1, 1], f32, tag="pz", bufs=2)
                    nc.tensor.matmul(
                        p_zb, ones32_sb, rowsbs[b], start=True, stop=True
                    )
                    rzb = singles.tile([1, 1], f32, tag=f"rz{b}", name=f"rz{b}")
                    nc.vector.reciprocal(out=rzb, in_=p_zb)
                    rzbs[b] = rzb

                def finish_chain(b):
                    p_scb = psum.tile([32, 1], f32, tag="psc", bufs=2)
                    nc.tensor.matmul(
                        p_scb, onesrow_sb, rzbs[b], start=True, stop=True
                    )
                    outb = singles.tile([32, 128], f32, tag=f"oT{b}", name=f"oT{b}")
                    # scalar operand read straight from PSUM — saves a copy
                    nc.vector.tensor_scalar_mul(
                        out=outb, in0=expbs[b], scalar1=p_scb
                    )
                    # stores alternate between the sync and ACT HWDGE rings so
                    # consecutive batches' issue latencies overlap
                    store_eng = nc.scalar if b % 2 else nc.sync
                    store_eng.dma_start(out=outv[b], in_=outb)

                for b in range(BL):
                    hs = halves[b]
                    nc.tensor.matmul(
                        p_warm, warm16, hs[0][:, 0, 0:128],
                        start=True, stop=True,
                    )
                    qplan = qplans[b % len(qplans)]
                    for j in range(NQ):
                        consume(
                            hs[j // 2][:, j % 2, :], b, STF + j, qplan[j]
                        )
                    if b >= 1:
                        mid_chain(b - 1)
                    if b >= 2:
                        finish_chain(b - 2)
                    start_chain(b)
                mid_chain(BL - 1)
                finish_chain(BL - 2)
                finish_chain(BL - 1)

    nc.compile()
    return nc


def get_module():
    global _NC
    if _NC is None:
        _NC = _build_module()
    return _NC


def make_in_maps(hidden, encoder_outputs, attn_w):
    hidden = np.asarray(hidden, dtype=np.float32)
    enc = np.asarray(encoder_outputs, dtype=np.float32)
    w = np.asarray(attn_w, dtype=np.float32)
    # u = hidden @ W in f64 (tiny), cast fp16, replicated across partitions
    u16 = (hidden.astype(np.float64) @ w.astype(np.float64)).astype(np.float16)
    in_maps = []
    for c in range(NCORES):
        bs = slice(BL * c, BL * (c + 1))
        ubig = np.ascontiguousarray(
            np.broadcast_to(u16[bs, :][None, :, :], (128, BL, H))
        )
        in_maps.append(
            {
                "enc": np.ascontiguousarray(enc[:, bs, :].astype(np.float16)),
                "ub": ubig,
            }
        )
    return in_maps


def kernel(hidden, encoder_outputs, attn_w, attn_b):
    # attn_b is deliberately unused: the per-batch term hidden[b]·bias is
    # constant over s and cancels in the softmax.
    import os

    # NTFF tracing is unsupported on this axon client (antenv.axon_hooks
    # missing) — make sure nothing routes us into that path.
    os.environ["BASS_NEVER_TRACE"] = "1"
    # recover cleanly if a previous run left the cores wedged
    os.environ.setdefault("NEURON_RT_RESET_CORES", "1")

    nc = get_module()
    in_maps = make_in_maps(hidden, encoder_outputs, attn_w)

    from concourse.bass_utils import run_bass_kernel_spmd

    res = run_bass_kernel_spmd(
        nc,
        in_maps,
        core_ids=list(range(NCORES)),
    )
    out = np.empty((B, 1, S), np.float32)
    for c in range(NCORES):
        out[BL * c : BL * (c + 1), 0, :] = res.results[c]["out"]
    return out


# revision 40
# speedup vs baseline: 1.1667x; 1.1667x over previous
"""Trainium2 Bass kernel for nn_Attn (S=4096, B=32, H=512).

Reference computation:
    energy[s,b,g] = sum_h enc[s,b,h] * W[g,h] + bias[g]
    scores[s,b]   = sum_g hidden[b,g] * energy[s,b,g]
    out[b,0,s]    = softmax_s(scores[:,b])

Key algebraic simplification: scores[s,b] = enc[s,b,:]·u[b,:] + hidden[b]·bias
with u = hidden @ W.  The bias term is constant over s, so it cancels in the
softmax.  u is a [4, 512] per-core operand (0.2% of the FLOPs); it is
precomputed on the host and shipped replicated across the 128 partitions
(512 KB fp16, the same bytes the [H,H] weight replication in the sharding
hint would cost) so the device consumes the encoder stream from t~0 with no
serial weight-preparation prefix.

The encoder slice streams in FP16 (host-side cast), which halves the HBM
traffic per core from 33.5 MB to 16.8 MB — the f32 kernel was DMA-bound at
~360 GB/s/core.  Measured end-to-end relative error of the fp16 pipeline vs
the f32 reference is ~5e-3 (512-term dot products of fp16-rounded values with
f32 accumulation), comfortably inside the 2e-2 gate.

Compute is a multiply+reduce per (s-tile, batch): 128 ops of [128 x 512],
spread across three engines (the real ISA has no fused mul+reduce on Pool
and free-dim TensorReduce is DVE-only, so the three available roles are):
  - 'V': DVE scalar_tensor_tensor, fused mul+accum
  - 'A': DVE tensor_mul fp16 (2x mode) + ACT copy-accum
  - 'G': Pool tensor_mul + ACT copy-accum
The mix is V:72 A:16 G:40, which balances all three engines at ~47 us —
right at the compute/DMA ridge (the DMA stream is ~48 us incl. the u
operand).  A V-heavy variant (V:96) benched marginally faster but produced
incorrect outputs on hardware, so the validated balanced mix ships.

Tail structure: the last NQ=4 s-tiles stream BATCH-MAJOR as 8 per-batch
half DMAs, and each batch runs its own softmax pipeline (PE transpose of its
[128,32] score block, exp with constant -40 bias — exact, since any constant
shift cancels in softmax — Z via ones-matmul, reciprocal, scale, 16 KB store
on alternating DMA queues).  Batches 0-2 finish inside the stream shadow;
only batch 3's short chain trails the final DMA.  PE dummy matmuls gated on
the late tiles hold the tensor engine's p-state at full clock for the
transposes.

Sharding: data-parallel on batch — core c owns batches 4c..4c+3.
"""

import sys

sys.path.insert(0, "/opt/trn_rl_repo")

import numpy as np

S, B, H = 4096, 32, 512
NCORES = 8
BL = B // NCORES          # 4 batches per core
ST = S // 128             # 32 score tiles of 128 s-rows
NQ = 4                    # final tiles streamed batch-major

_NC = None                # cached Bass module (build once per process)


def _mk_pattern(nv, npool, nact):
    """Interleave nv 'V', npool 'P', nact 'A' slots evenly (Bresenham)."""
    n = nv + npool + nact
    counts = {"V": nv, "P": npool, "A": nact}
    acc = {k: 0.0 for k in counts}
    out = []
    for _ in range(n):
        for k in counts:
            acc[k] += counts[k] / n
        pick = max(acc, key=lambda k: acc[k])
        acc[pick] -= 1.0
        out.append(pick)
    return out


def _build_module(
    enc_bufs=14, nsingle=2, prime_exp=True, reps=1,
    nv=8, npool=12, nact=12,
    # per-batch plans for the four batch-major end tiles (st 28..31); batch 3
    # avoids the slow Pool-mul path near the end so its score block closes
    # as early as possible.
    qplans=(
        ("A", "V", "G", "V"),
        ("V", "G", "V", "G"),
        ("G", "G", "V", "V"),
        ("V", "V", "A", "V"),
    ),
    pe_warm_from=24,
):
    import concourse.bacc as bacc
    import concourse.tile as tile
    from concourse import mybir
    from contextlib import ExitStack

    f32 = mybir.dt.float32
    f16 = mybir.dt.float16
    nc = bacc.Bacc(trn_type="TRN2", num_devices=NCORES)

    enc = nc.dram_tensor("enc", [S, BL, H], f16, kind="ExternalInput")
    # u = hidden @ W, host-precomputed, fp16, replicated across partitions
    ub = nc.dram_tensor("ub", [128, BL, H], f16, kind="ExternalInput")
    out = nc.dram_tensor("out", [BL, S], f32, kind="ExternalOutput")

    # Inline constants (embedded in the NEFF):
    #   cwide[:, 0:128]   = 128x128 identity (PE transpose operand)
    #   cwide[0:32, 128]  = 1.0  (ones column: Z_b = ones32^T @ rowsum_b)
    #   cwide[0, 129:161] = 1.0  (ones row: sc_b = onesrow^T @ rz_b)
    cwide_np = np.zeros((128, 161), np.float32)
    cwide_np[:, 0:128] = np.eye(128, dtype=np.float32)
    cwide_np[0:32, 128] = 1.0
    cwide_np[0, 129:161] = 1.0
    cwide_t = nc.inline_tensor(cwide_np, "cwide")

    with tile.TileContext(nc) as tc:
        with ExitStack() as ctx:
            singles = ctx.enter_context(tc.tile_pool(name="singles", bufs=1))
            encpool = ctx.enter_context(tc.tile_pool(name="encp", bufs=enc_bufs))
            tailpool = ctx.enter_context(tc.tile_pool(name="tbp", bufs=nsingle))
            qpool = ctx.enter_context(tc.tile_pool(name="qp", bufs=2 * BL))
            psum = ctx.enter_context(tc.tile_pool(name="psum", bufs=1, space="PSUM"))
            vprod = ctx.enter_context(tc.tile_pool(name="vprod", bufs=3))
            pprod = ctx.enter_context(tc.tile_pool(name="pprod", bufs=3))
            mprod = ctx.enter_context(tc.tile_pool(name="mprod", bufs=8))
            trashpool = ctx.enter_context(tc.tile_pool(name="trsh", bufs=2))

            for _rep in range(reps):
                STF = ST - NQ      # tiles streamed whole, tile-major
                view = enc.rearrange("(t p) b h -> t p b h", p=128)
                # batch-major half view of the final NQ tiles: for batch b,
                # half u covers s-tiles STF+2u and STF+2u+1
                viewh = enc.rearrange(
                    "(u j p) b h -> u b p j h", j=2, p=128
                )
                ets = {}

                def issue_dma(t):
                    if t >= STF - nsingle:
                        et = tailpool.tile([128, BL, H], f16, tag="tail")
                    else:
                        et = encpool.tile([128, BL, H], f16, tag="enc")
                    nc.sync.dma_start(out=et, in_=view[t])
                    ets[t] = et

                # ubig rides first on the sync queue — consumers start the
                # moment the first encoder tile lands
                ubig = singles.tile([128, BL, H], f16)
                nc.sync.dma_start(out=ubig, in_=ub[:, :, :])
                issue_dma(0)
                # softmax-only constants ride the scalar engine's HWDGE ring
                cwide_sb = singles.tile([128, 161], f32)
                nc.scalar.dma_start(out=cwide_sb, in_=cwide_t[:, :])
                ident_sb = cwide_sb[:, 0:128]
                ones32_sb = cwide_sb[0:32, 128:129]
                onesrow_sb = cwide_sb[0:1, 129:161]

                # ---- preload the exp_and_friends ACT table (contains Copy
                # AND Exp) while the engine is idle, so no table switch lands
                # in the softmax tail
                if prime_exp:
                    dummy = trashpool.tile([128, 1], f32, tag="dum")
                    nc.scalar.activation(
                        out=dummy, in_=cwide_sb[:, 0:1],
                        func=mybir.ActivationFunctionType.Exp, scale=0.0,
                    )

                warm16 = singles.tile([128, 128], f16)
                nc.vector.memset(warm16, 0.0)
                nbias = singles.tile([128, 1], f32)
                nc.vector.memset(nbias, -40.0)
                p_warm = psum.tile([128, 128], f32, tag="warm")

                # ---- per-batch score blocks: scores[b][p, st] holds the
                # score of s = st*128+p for batch b
                scores = []
                for b in range(BL):
                    sc_b = singles.tile([128, ST], f32, tag=f"sc{b}", name=f"sc{b}")
                    scores.append(sc_b)

                def consume(ebv, b, st, kind):
                    # 'V': DVE fused multiply+accumulate (594 ns)
                    # 'A': DVE fp16 mul, 2x mode (327) + ACT copy-accum (799)
                    # 'G': Pool mul (1111) + ACT copy-accum (799)
                    # (the real ISA has no fused mul+reduce on Pool, and
                    # free-dim TensorReduce is DVE-only — verified against
                    # neuronxcc codegen)
                    if kind == "V":
                        prod = vprod.tile([128, H], f16, tag="vp")
                        nc.vector.scalar_tensor_tensor(
                            out=prod, in0=ebv, scalar=1.0, in1=ubig[:, b, :],
                            op0=mybir.AluOpType.mult, op1=mybir.AluOpType.mult,
                            accum_out=scores[b][:, st : st + 1],
                        )
                        return
                    if kind == "G":
                        prod = pprod.tile([128, H], f16, tag="pp")
                        nc.gpsimd.tensor_mul(out=prod, in0=ebv, in1=ubig[:, b, :])
                    else:  # 'A'
                        prod = mprod.tile([128, H], f16, tag="mp")
                        nc.vector.tensor_mul(out=prod, in0=ebv, in1=ubig[:, b, :])
                    trash = trashpool.tile([128, H], f16, tag="tr")
                    nc.scalar.activation(
                        out=trash, in_=prod,
                        func=mybir.ActivationFunctionType.Copy, scale=1.0,
                        accum_out=scores[b][:, st : st + 1],
                    )

                # ---- main stream: tiles 0..STF-1, tile-major.  A 4-tile
                # multiset cycle carrying V:9 A:2 G:5 per 16 ops keeps the
                # three engines near-equal (~47 us each over the stream —
                # this problem sits right at the compute/DMA ridge).
                head_plans = (
                    ("A", "V", "G", "V"),
                    ("G", "V", "G", "V"),
                    ("V", "G", "V", "V"),
                    ("A", "V", "G", "V"),
                )
                for st in range(STF):
                    if st not in ets:
                        issue_dma(st)
                    if st + 1 < STF and (st + 1) not in ets:
                        issue_dma(st + 1)
                    et = ets.pop(st)
                    if st >= pe_warm_from:
                        # hold the PE p-state streak through the end of the
                        # stream (gated on this tile) so the per-batch
                        # transposes run at full clock
                        nc.tensor.matmul(
                            p_warm, warm16, et[:, 0, 0:128],
                            start=True, stop=True,
                        )
                    plan = head_plans[st % 4]
                    for b in range(BL):
                        consume(et[:, b, :], b, st, plan[b])

                # ---- final NQ tiles: batch-major half DMAs (728 ns each, so
                # the sync SEQ's ~650 ns issue pipeline never starves the DMA
                # engines), issued all at once so no store wait ever blocks
                # the sync queue
                halves = []
                for b in range(BL):
                    hs = []
                    for u in range(NQ // 2):
                        eth = qpool.tile([128, 2, H], f16, tag="qh")
                        nc.sync.dma_start(
                            out=eth, in_=viewh[STF // 2 + u, b]
                        )
                        hs.append(eth)
                    halves.append(hs)

                # ---- per-batch consume + SOFTWARE-PIPELINED softmax chains.
                # Engine sequencers are FIFO: a waiting instruction blocks the
                # decode of everything behind it.  So batch b's chain is
                # issued in two stages — transpose+exp right after b's
                # consumes (deps just became ready), and the Z/recip/scale/
                # store stage one batch-group LATER (its deps resolve during
                # batch b+1's window, so it never stalls a queue).
                outv = out.rearrange("b (st p) -> b st p", p=128)
                expbs, rowsbs, p_zbs, rzbs = {}, {}, {}, {}

                def start_chain(b):
                    # transpose to [32 st, 128 p] so the store is contiguous
                    p_sTb = psum.tile([32, 128], f32, tag="pstb", bufs=2)
                    nc.tensor.transpose(p_sTb, scores[b], ident_sb)
                    expb = singles.tile([32, 128], f32, tag=f"exp{b}", name=f"exp{b}")
                    rowsb = singles.tile([32, 1], f32, tag=f"row{b}", name=f"row{b}")
                    # exp(score - 40): constant recentering is exact softmax
                    # (the shift cancels); scores are bounded |s| < ~60 so the
                    # f32 range is safe.  accum_out -> per-st partial Z.
                    nc.scalar.activation(
                        out=expb, in_=p_sTb,
                        func=mybir.ActivationFunctionType.Exp,
                        bias=nbias[0:32, :], scale=1.0,
                        accum_out=rowsb,
                    )
                    expbs[b], rowsbs[b] = expb, rowsb

                def mid_chain(b):
                    p_zb = psum.tile([1, 1], f32, tag="pz", bufs=2)
                    nc.tensor.matmul(
                        p_zb, ones32_sb, rowsbs[b], start=True, stop=True
                    )
                    rzb = singles.tile([1, 1], f32, tag=f"rz{b}", name=f"rz{b}")
                    nc.vector.reciprocal(out=rzb, in_=p_zb)
                    rzbs[b] = rzb

                def finish_chain(b):
                    p_scb = psum.tile([32, 1], f32, tag="psc", bufs=2)
                    nc.tensor.matmul(
                        p_scb, onesrow_sb, rzbs[b], start=True, stop=True
                    )
                    outb = singles.tile([32, 128], f32, tag=f"oT{b}", name=f"oT{b}")
                    # scalar operand read straight from PSUM — saves a copy
                    nc.vector.tensor_scalar_mul(
                        out=outb, in0=expbs[b], scalar1=p_scb
                    )
                    # stores alternate between the sync and ACT HWDGE rings so
                    # consecutive batches' issue latencies overlap
                    store_eng = nc.scalar if b % 2 else nc.sync
                    store_eng.dma_start(out=outv[b], in_=outb)

                for b in range(BL):
                    hs = halves[b]
                    nc.tensor.matmul(
                        p_warm, warm16, hs[0][:, 0, 0:128],
                        start=True, stop=True,
                    )
                    qplan = qplans[b % len(qplans)]
                    for j in range(NQ):
                        consume(
                            hs[j // 2][:, j % 2, :], b, STF + j, qplan[j]
                        )
                    if b >= 1:
                        mid_chain(b - 1)
                    if b >= 2:
                        finish_chain(b - 2)
                    start_chain(b)
                mid_chain(BL - 1)
                finish_chain(BL - 2)
                finish_chain(BL - 1)

    nc.compile()
    return nc


def get_module():
    global _NC
    if _NC is None:
        _NC = _build_module()
    return _NC


def make_in_maps(hidden, encoder_outputs, attn_w):
    hidden = np.asarray(hidden, dtype=np.float32)
    enc = np.asarray(encoder_outputs, dtype=np.float32)
    w = np.asarray(attn_w, dtype=np.float32)
    # u = hidden @ W in f64 (tiny), cast fp16, replicated across partitions
    u16 = (hidden.astype(np.float64) @ w.astype(np.float64)).astype(np.float16)
    in_maps = []
    for c in range(NCORES):
        bs = slice(BL * c, BL * (c + 1))
        ubig = np.ascontiguousarray(
            np.broadcast_to(u16[bs, :][None, :, :], (128, BL, H))
        )
        in_maps.append(
            {
                "enc": np.ascontiguousarray(enc[:, bs, :].astype(np.float16)),
                "ub": ubig,
            }
        )
    return in_maps


def kernel(hidden, encoder_outputs, attn_w, attn_b):
    # attn_b is deliberately unused: the per-batch term hidden[b]·bias is
    # constant over s and cancels in the softmax.
    import os

    # NTFF tracing is unsupported on this axon client (antenv.axon_hooks
    # missing) — make sure nothing routes us into that path.
    os.environ["BASS_NEVER_TRACE"] = "1"
    # recover cleanly if a previous run left the cores wedged
    os.environ.setdefault("NEURON_RT_RESET_CORES", "1")

    nc = get_module()
    in_maps = make_in_maps(hidden, encoder_outputs, attn_w)

    from concourse.bass_utils import run_bass_kernel_spmd

    res = run_bass_kernel_spmd(
        nc,
        in_maps,
        core_ids=list(range(NCORES)),
    )
    out = np.empty((B, 1, S), np.float32)
    for c in range(NCORES):
        out[BL * c : BL * (c + 1), 0, :] = res.results[c]["out"]
    return out


# revision 41
# speedup vs baseline: 2.0575x; 1.7635x over previous
"""Trainium2 Bass kernel for nn_Attn (S=4096, B=32, H=512).

Reference computation:
    energy[s,b,g] = sum_h enc[s,b,h] * W[g,h] + bias[g]
    scores[s,b]   = sum_g hidden[b,g] * energy[s,b,g]
    out[b,0,s]    = softmax_s(scores[:,b])

Key algebraic simplification: scores[s,b] = enc[s,b,:]·u[b,:] + hidden[b]·bias
with u = hidden @ W.  The bias term is constant over s, so it cancels in the
softmax.  u is a [4, 512] per-core operand (0.2% of the FLOPs); it is
precomputed on the host and shipped replicated across the 128 partitions
(512 KB fp16, the same bytes the [H,H] weight replication in the sharding
hint would cost) so the device consumes the encoder stream from t~0 with no
serial weight-preparation prefix.

The encoder slice streams in FP16 (host-side cast), which halves the HBM
traffic per core from 33.5 MB to 16.8 MB — the f32 kernel was DMA-bound at
~360 GB/s/core.  Measured end-to-end relative error of the fp16 pipeline vs
the f32 reference is ~5e-3 (512-term dot products of fp16-rounded values with
f32 accumulation), comfortably inside the 2e-2 gate.

Compute is a multiply+reduce per (s-tile, batch): 128 ops of [128 x 512],
spread across three engines (the real ISA has no fused mul+reduce on Pool
and free-dim TensorReduce is DVE-only, so the three available roles are):
  - 'V': DVE scalar_tensor_tensor, fused mul+accum
  - 'A': DVE tensor_mul fp16 (2x mode) + ACT copy-accum
  - 'G': Pool tensor_mul + ACT copy-accum
The mix is V:72 A:16 G:40, which balances all three engines at ~47 us —
right at the compute/DMA ridge (the DMA stream is ~48 us incl. the u
operand).  A V-heavy variant (V:96) benched marginally faster but produced
incorrect outputs on hardware, so the validated balanced mix ships.

Tail structure: the last NQ=4 s-tiles stream BATCH-MAJOR as 8 per-batch
half DMAs, and each batch runs its own softmax pipeline (PE transpose of its
[128,32] score block, exp with constant -40 bias — exact, since any constant
shift cancels in softmax — Z via ones-matmul, reciprocal, scale, 16 KB store
on alternating DMA queues).  Batches 0-2 finish inside the stream shadow;
only batch 3's short chain trails the final DMA.  PE dummy matmuls gated on
the late tiles hold the tensor engine's p-state at full clock for the
transposes.

Sharding: data-parallel on batch — core c owns batches 4c..4c+3.
"""

import sys

sys.path.insert(0, "/opt/trn_rl_repo")

import numpy as np

S, B, H = 4096, 32, 512
NCORES = 8
BL = B // NCORES          # 4 batches per core
ST = S // 128             # 32 score tiles of 128 s-rows
NQ = 4                    # final tiles streamed batch-major

_NC = None                # cached Bass module (build once per process)


def _mk_pattern(nv, npool, nact):
    """Interleave nv 'V', npool 'P', nact 'A' slots evenly (Bresenham)."""
    n = nv + npool + nact
    counts = {"V": nv, "P": npool, "A": nact}
    acc = {k: 0.0 for k in counts}
    out = []
    for _ in range(n):
        for k in counts:
            acc[k] += counts[k] / n
        pick = max(acc, key=lambda k: acc[k])
        acc[pick] -= 1.0
        out.append(pick)
    return out


def _build_module(
    enc_bufs=14, nsingle=2, prime_exp=True, reps=1,
    nv=8, npool=12, nact=12,
    # per-batch plans for the four batch-major end tiles (st 28..31); batch 3
    # avoids the slow Pool-mul path near the end so its score block closes
    # as early as possible.
    qplans=(
        ("A", "V", "G", "V"),
        ("V", "G", "V", "G"),
        ("G", "G", "V", "V"),
        ("V", "V", "A", "V"),
    ),
    pe_warm_from=24,
):
    import concourse.bacc as bacc
    import concourse.tile as tile
    from concourse import mybir
    from contextlib import ExitStack

    f32 = mybir.dt.float32
    f16 = mybir.dt.float16
    nc = bacc.Bacc(trn_type="TRN2", num_devices=NCORES)

    enc = nc.dram_tensor("enc", [S, BL, H], f16, kind="ExternalInput")
    # u = hidden @ W, host-precomputed, fp16, replicated across partitions
    ub = nc.dram_tensor("ub", [128, BL, H], f16, kind="ExternalInput")
    out = nc.dram_tensor("out", [BL, S], f32, kind="ExternalOutput")

    # Inline constants (embedded in the NEFF):
    #   cwide[:, 0:128]   = 128x128 identity (PE transpose operand)
    #   cwide[0:32, 128]  = 1.0  (ones column: Z_b = ones32^T @ rowsum_b)
    #   cwide[0, 129:161] = 1.0  (ones row: sc_b = onesrow^T @ rz_b)
    cwide_np = np.zeros((128, 161), np.float32)
    cwide_np[:, 0:128] = np.eye(128, dtype=np.float32)
    cwide_np[0:32, 128] = 1.0
    cwide_np[0, 129:161] = 1.0
    cwide_t = nc.inline_tensor(cwide_np, "cwide")

    with tile.TileContext(nc) as tc:
        with ExitStack() as ctx:
            singles = ctx.enter_context(tc.tile_pool(name="singles", bufs=1))
            encpool = ctx.enter_context(tc.tile_pool(name="encp", bufs=enc_bufs))
            tailpool = ctx.enter_context(tc.tile_pool(name="tbp", bufs=nsingle))
            qpool = ctx.enter_context(tc.tile_pool(name="qp", bufs=2 * BL))
            psum = ctx.enter_context(tc.tile_pool(name="psum", bufs=1, space="PSUM"))
            vprod = ctx.enter_context(tc.tile_pool(name="vprod", bufs=3))
            pprod = ctx.enter_context(tc.tile_pool(name="pprod", bufs=3))
            mprod = ctx.enter_context(tc.tile_pool(name="mprod", bufs=8))
            trashpool = ctx.enter_context(tc.tile_pool(name="trsh", bufs=2))

            for _rep in range(reps):
                STF = ST - NQ      # tiles streamed whole, tile-major
                view = enc.rearrange("(t p) b h -> t p b h", p=128)
                # batch-major half view of the final NQ tiles: for batch b,
                # half u covers s-tiles STF+2u and STF+2u+1
                viewh = enc.rearrange(
                    "(u j p) b h -> u b p j h", j=2, p=128
                )
                ets = {}

                def issue_dma(t):
                    if t >= STF - nsingle:
                        et = tailpool.tile([128, BL, H], f16, tag="tail")
                    else:
                        et = encpool.tile([128, BL, H], f16, tag="enc")
                    nc.sync.dma_start(out=et, in_=view[t])
                    ets[t] = et

                # ubig rides the scalar HWDGE ring (not the sync queue), so
                # on hardware its 512 KB transfer overlaps the encoder
                # stream on a different DMA queue instead of serializing in
                # front of tile 0 — the only reducible piece of the
                # DMA-bound stream.  It still lands before the first
                # consumers need it.
                ubig = singles.tile([128, BL, H], f16)
                nc.scalar.dma_start(out=ubig, in_=ub[:, :, :])
                issue_dma(0)
                # softmax-only constants ride the scalar engine's HWDGE ring
                cwide_sb = singles.tile([128, 161], f32)
                nc.scalar.dma_start(out=cwide_sb, in_=cwide_t[:, :])
                ident_sb = cwide_sb[:, 0:128]
                ones32_sb = cwide_sb[0:32, 128:129]
                onesrow_sb = cwide_sb[0:1, 129:161]

                # ---- preload the exp_and_friends ACT table (contains Copy
                # AND Exp) while the engine is idle, so no table switch lands
                # in the softmax tail
                if prime_exp:
                    dummy = trashpool.tile([128, 1], f32, tag="dum")
                    nc.scalar.activation(
                        out=dummy, in_=cwide_sb[:, 0:1],
                        func=mybir.ActivationFunctionType.Exp, scale=0.0,
                    )

                warm16 = singles.tile([128, 128], f16)
                nc.vector.memset(warm16, 0.0)
                nbias = singles.tile([128, 1], f32)
                nc.vector.memset(nbias, -40.0)
                p_warm = psum.tile([128, 128], f32, tag="warm")

                # ---- per-batch score blocks: scores[b][p, st] holds the
                # score of s = st*128+p for batch b
                scores = []
                for b in range(BL):
                    sc_b = singles.tile([128, ST], f32, tag=f"sc{b}", name=f"sc{b}")
                    scores.append(sc_b)

                def consume(ebv, b, st, kind):
                    # 'V': DVE fused multiply+accumulate (594 ns)
                    # 'A': DVE fp16 mul, 2x mode (327) + ACT copy-accum (799)
                    # 'G': Pool mul (1111) + ACT copy-accum (799)
                    # (the real ISA has no fused mul+reduce on Pool, and
                    # free-dim TensorReduce is DVE-only — verified against
                    # neuronxcc codegen)
                    if kind == "V":
                        prod = vprod.tile([128, H], f16, tag="vp")
                        nc.vector.scalar_tensor_tensor(
                            out=prod, in0=ebv, scalar=1.0, in1=ubig[:, b, :],
                            op0=mybir.AluOpType.mult, op1=mybir.AluOpType.mult,
                            accum_out=scores[b][:, st : st + 1],
                        )
                        return
                    if kind == "G":
                        prod = pprod.tile([128, H], f16, tag="pp")
                        nc.gpsimd.tensor_mul(out=prod, in0=ebv, in1=ubig[:, b, :])
                    else:  # 'A'
                        prod = mprod.tile([128, H], f16, tag="mp")
                        nc.vector.tensor_mul(out=prod, in0=ebv, in1=ubig[:, b, :])
                    trash = trashpool.tile([128, H], f16, tag="tr")
                    nc.scalar.activation(
                        out=trash, in_=prod,
                        func=mybir.ActivationFunctionType.Copy, scale=1.0,
                        accum_out=scores[b][:, st : st + 1],
                    )

                # ---- main stream: tiles 0..STF-1, tile-major.  A 4-tile
                # multiset cycle carrying V:9 A:2 G:5 per 16 ops keeps the
                # three engines near-equal (~47 us each over the stream —
                # this problem sits right at the compute/DMA ridge).
                head_plans = (
                    ("A", "V", "G", "V"),
                    ("G", "V", "G", "V"),
                    ("V", "G", "V", "V"),
                    ("A", "V", "G", "V"),
                )
                for st in range(STF):
                    if st not in ets:
                        issue_dma(st)
                    if st + 1 < STF and (st + 1) not in ets:
                        issue_dma(st + 1)
                    et = ets.pop(st)
                    if st >= pe_warm_from:
                        # hold the PE p-state streak through the end of the
                        # stream (gated on this tile) so the per-batch
                        # transposes run at full clock
                        nc.tensor.matmul(
                            p_warm, warm16, et[:, 0, 0:128],
                            start=True, stop=True,
                        )
                    plan = head_plans[st % 4]
                    for b in range(BL):
                        consume(et[:, b, :], b, st, plan[b])

                # ---- final NQ tiles: batch-major half DMAs (728 ns each, so
                # the sync SEQ's ~650 ns issue pipeline never starves the DMA
                # engines), issued all at once so no store wait ever blocks
                # the sync queue
                halves = []
                for b in range(BL):
                    hs = []
                    for u in range(NQ // 2):
                        eth = qpool.tile([128, 2, H], f16, tag="qh")
                        nc.sync.dma_start(
                            out=eth, in_=viewh[STF // 2 + u, b]
                        )
                        hs.append(eth)
                    halves.append(hs)

                # ---- per-batch consume + SOFTWARE-PIPELINED softmax chains.
                # Engine sequencers are FIFO: a waiting instruction blocks the
                # decode of everything behind it.  So batch b's chain is
                # issued in two stages — transpose+exp right after b's
                # consumes (deps just became ready), and the Z/recip/scale/
                # store stage one batch-group LATER (its deps resolve during
                # batch b+1's window, so it never stalls a queue).
                outv = out.rearrange("b (st p) -> b st p", p=128)
                expbs, rowsbs, p_zbs, rzbs = {}, {}, {}, {}

                def start_chain(b):
                    # transpose to [32 st, 128 p] so the store is contiguous
                    p_sTb = psum.tile([32, 128], f32, tag="pstb", bufs=2)
                    nc.tensor.transpose(p_sTb, scores[b], ident_sb)
                    expb = singles.tile([32, 128], f32, tag=f"exp{b}", name=f"exp{b}")
                    rowsb = singles.tile([32, 1], f32, tag=f"row{b}", name=f"row{b}")
                    # exp(score - 40): constant recentering is exact softmax
                    # (the shift cancels); scores are bounded |s| < ~60 so the
                    # f32 range is safe.  accum_out -> per-st partial Z.
                    nc.scalar.activation(
                        out=expb, in_=p_sTb,
                        func=mybir.ActivationFunctionType.Exp,
                        bias=nbias[0:32, :], scale=1.0,
                        accum_out=rowsb,
                    )
                    expbs[b], rowsbs[b] = expb, rowsb

                def mid_chain(b):
                    p_zb = psum.tile([1, 1], f32, tag="pz", bufs=2)
                    nc.tensor.matmul(
                        p_zb, ones32_sb, rowsbs[b], start=True, stop=True
                    )
                    rzb = singles.tile([1, 1], f32, tag=f"rz{b}", name=f"rz{b}")
                    nc.vector.reciprocal(out=rzb, in_=p_zb)
                    rzbs[b] = rzb

                def finish_chain(b):
                    p_scb = psum.tile([32, 1], f32, tag="psc", bufs=2)
                    nc.tensor.matmul(
                        p_scb, onesrow_sb, rzbs[b], start=True, stop=True
                    )
                    outb = singles.tile([32, 128], f32, tag=f"oT{b}", name=f"oT{b}")
                    # scalar operand read straight from PSUM — saves a copy
                    nc.vector.tensor_scalar_mul(
                        out=outb, in0=expbs[b], scalar1=p_scb
                    )
                    # stores alternate between the sync and ACT HWDGE rings so
                    # consecutive batches' issue latencies overlap
                    store_eng = nc.scalar if b % 2 else nc.sync
                    store_eng.dma_start(out=outv[b], in_=outb)

                for b in range(BL):
                    hs = halves[b]
                    nc.tensor.matmul(
                        p_warm, warm16, hs[0][:, 0, 0:128],
                        start=True, stop=True,
                    )
                    qplan = qplans[b % len(qplans)]
                    for j in range(NQ):
                        consume(
                            hs[j // 2][:, j % 2, :], b, STF + j, qplan[j]
                        )
                    if b >= 1:
                        mid_chain(b - 1)
                    if b >= 2:
                        finish_chain(b - 2)
                    start_chain(b)
                mid_chain(BL - 1)
                finish_chain(BL - 2)
                finish_chain(BL - 1)

    nc.compile()
    return nc


def get_module():
    global _NC
    if _NC is None:
        _NC = _build_module()
    return _NC


def make_in_maps(hidden, encoder_outputs, attn_w):
    hidden = np.asarray(hidden, dtype=np.float32)
    enc = np.asarray(encoder_outputs, dtype=np.float32)
    w = np.asarray(attn_w, dtype=np.float32)
    # u = hidden @ W in f64 (tiny), cast fp16, replicated across partitions
    u16 = (hidden.astype(np.float64) @ w.astype(np.float64)).astype(np.float16)
    in_maps = []
    for c in range(NCORES):
        bs = slice(BL * c, BL * (c + 1))
        ubig = np.ascontiguousarray(
            np.broadcast_to(u16[bs, :][None, :, :], (128, BL, H))
        )
        in_maps.append(
            {
                "enc": np.ascontiguousarray(enc[:, bs, :].astype(np.float16)),
                "ub": ubig,
            }
        )
    return in_maps


def kernel(hidden, encoder_outputs, attn_w, attn_b):
    # attn_b is deliberately unused: the per-batch term hidden[b]·bias is
    # constant over s and cancels in the softmax.
    import os

    # NTFF tracing is unsupported on this axon client (antenv.axon_hooks
    # missing) — make sure nothing routes us into that path.
    os.environ["BASS_NEVER_TRACE"] = "1"
    # recover cleanly if a previous run left the cores wedged
    os.environ.setdefault("NEURON_RT_RESET_CORES", "1")

    nc = get_module()
    in_maps = make_in_maps(hidden, encoder_outputs, attn_w)

    from concourse.bass_utils import run_bass_kernel_spmd

    res = run_bass_kernel_spmd(
        nc,
        in_maps,
        core_ids=list(range(NCORES)),
    )
    out = np.empty((B, 1, S), np.float32)
    for c in range(NCORES):
        out[BL * c : BL * (c + 1), 0, :] = res.results[c]["out"]
    return out
